# revision 1
# baseline (speedup 1.0000x reference)
"""Trainium2 Bass kernel for nn_CrossAttention_60129542144729.

Data-parallel over batch B=8 across 8 NeuronCores (one batch element per
core). Each core runs the full fused forward in channel-major layout:

  GroupNorm+SiLU -> 3x3x3 conv (as 27 shifted matmuls on a zero-padded
  18^3 volume, with the attention q-projection wq folded into the conv
  weights on the host) -> masked cross-attention in score-transposed
  layout (mask built from integer pixel arithmetic on device) -> output
  projection.

All heavy matmuls run in bf16 with fp32 PSUM accumulation (measured
end-to-end l2 relative error ~3e-3 vs the fp32 reference).
"""
import math
from contextlib import ExitStack

import numpy as np
import ml_dtypes

import concourse.bass as bass
import concourse.mybir as mybir
import concourse.tile as tile
from concourse import bacc
from concourse.masks import make_identity

BF16 = ml_dtypes.bfloat16
f32 = mybir.dt.float32
bf16 = mybir.dt.bfloat16

# ---------------------------------------------------------------- constants
F = 512
H = 8
IMG = 16
L = IMG ** 3    # 4096
S = 196         # patch number
VIT_RES = 14
SKETCH = 768
B = 8
NLC = 8         # number of 512-column l-chunks
LC = 512
PADV = 18       # padded volume edge
PAD3 = PADV ** 3


def _position_encoding(d_model, length):
    pe = np.zeros((length, d_model), dtype=np.float32)
    position = np.arange(length, dtype=np.float32)[:, None]
    div_term = np.exp(np.arange(0, d_model, 2, dtype=np.float32)
                      * -(math.log(10000.0) / d_model))
    pe[:, 0::2] = np.sin(position * div_term)
    pe[:, 1::2] = np.cos(position * div_term)
    return pe


def _voxel_coordinates(resolution, size=1.0):
    idx = np.arange(resolution, dtype=np.float32)
    g = np.stack(np.meshgrid(idx, idx, idx, indexing="ij"), -1).reshape(-1, 3)
    return (g + 0.5) / resolution * size - size / 2.0


VOXEL_PE = _position_encoding(F, L)                    # (L, F)
COND_PE = _position_encoding(F, S)                     # (S, F)
VOXEL_PTS = _voxel_coordinates(IMG)                    # (L, 3)
_vg = np.arange(VIT_RES, dtype=np.float32)
VIT_PIXELS = np.stack(np.meshgrid(_vg, _vg, indexing="ij"), -1).reshape(S, 2)

# mask constants: masked-out iff (pyr - CX16[s])^2 + (pxr - CY16[s])^2 >= 576
CX16 = (16.0 * VIT_PIXELS[:, 0] - 16.0 * (1.0 / 32.0 - 0.5)).astype(np.float32)
CY16 = (-(16.0 * VIT_PIXELS[:, 1] - 16.0 * (447.0 / 32.0 - 0.5))).astype(np.float32)

SPARTS = [(0, 128), (128, 68)]   # s-dimension partition tiles
KSPLIT = 14                      # conv offset split for weight streaming


# ------------------------------------------------------------- the program
def _declare(nc):
    t = {}

    def din(name, shape, dt):
        t[name] = nc.dram_tensor(name, list(shape), dt, kind="ExternalInput").ap()

    din("x", (F, L), f32)
    din("wfold", (4, 128, 27 * 4 * 128), bf16)   # [oc][ic128][k*4ic*128oc]
    din("cq", (4, 128, L), bf16)
    din("sketchT", (SKETCH, S), bf16)
    din("wkT", (SKETCH, F), bf16)
    din("wvT", (SKETCH, F), bf16)
    din("ck", (F, S), bf16)
    din("cvT", (S, F), f32)
    din("woT", (F, F), bf16)
    din("outb", (F, 1), f32)
    din("gns", (F, 1), f32)
    din("gnb", (F, 1), f32)
    din("pmat", (3, 3), f32)       # P^T (rhs of the voxel-projection matmul)
    din("vptsT", (3, L), f32)
    din("cx16", (S, 1), f32)
    din("cy16", (S, 1), f32)
    din("gmat", (128, 8), f32)
    din("gmatT", (8, 128), f32)
    t["out"] = nc.dram_tensor("out", [F, L], f32, kind="ExternalOutput").ap()
    return t


def _build(ctx: ExitStack, tc: tile.TileContext, t):
    nc = tc.nc
    AT = mybir.ActivationFunctionType
    OP = mybir.AluOpType
    X = mybir.AxisListType.X

    singles = ctx.enter_context(tc.tile_pool(name="singles", bufs=1))
    dram = ctx.enter_context(tc.tile_pool(name="dram", bufs=1, space="DRAM"))

    dbg = "ExternalOutput" if DEBUG_DUMPS else "Internal"
    qhT_d = dram.tile([F, L], bf16, kind=dbg, name="qhT_d")
    oT_d = dram.tile([F, L], bf16, kind=dbg, name="oT_d")
    pflat_d = dram.tile([2, L], f32, kind=dbg, name="pflat_d")
    if DEBUG_DUMPS:
        mask_d = dram.tile([256, L], bf16, kind=dbg, name="mask_d")
        khT_dd = dram.tile([F, S], bf16, kind=dbg, name="khT_dd")
        vha_dd = dram.tile([S, 1024], bf16, kind=dbg, name="vha_dd")

    # ---------------- resident constants ----------------
    g_sb = singles.tile([128, 8], f32)
    nc.sync.dma_start(g_sb[:], t["gmat"])
    gT_sb = singles.tile([8, 128], f32)
    nc.sync.dma_start(gT_sb[:], t["gmatT"])
    gns_sb = singles.tile([128, 4], f32)
    gnb_sb = singles.tile([128, 4], f32)
    for i in range(4):
        nc.sync.dma_start(gns_sb[:, i:i + 1], t["gns"][128 * i:128 * i + 128])
        nc.sync.dma_start(gnb_sb[:, i:i + 1], t["gnb"][128 * i:128 * i + 128])
    ident = singles.tile([128, 128], f32)
    make_identity(nc, ident[:])
    eps_sb = singles.tile([128, 1], f32)
    nc.vector.memset(eps_sb[:], 1e-5)
    cx_sb = singles.tile([128, 2], f32)   # col0: s 0..127, col1: s 128..195
    cy_sb = singles.tile([128, 2], f32)
    for i, (so, sz) in enumerate(SPARTS):
        nc.sync.dma_start(cx_sb[:sz, i:i + 1], t["cx16"][so:so + sz])
        nc.sync.dma_start(cy_sb[:sz, i:i + 1], t["cy16"][so:so + sz])
    wo_sb = [singles.tile([128, F], bf16, tag=f"wo{j}", name=f"wo{j}")
             for j in range(4)]
    for j in range(4):
        nc.sync.dma_start(wo_sb[j][:], t["woT"][128 * j:128 * j + 128])
    outb_sb = singles.tile([128, 4], f32)
    for i in range(4):
        nc.sync.dma_start(outb_sb[:, i:i + 1], t["outb"][128 * i:128 * i + 128])

    hpad_pool = ctx.enter_context(tc.tile_pool(name="hpad", bufs=1))
    hpads = []
    for ic in range(4):
        hp = hpad_pool.tile([128, PAD3], bf16, tag=f"hpad{ic}", name=f"hpad{ic}")
        nc.vector.memset(hp[:], 0.0)
        hpads.append(hp)

    # ---------------- GroupNorm + SiLU -> padded volume ----------------
    with ExitStack() as phase:
        gnp = phase.enter_context(tc.tile_pool(name="gn", bufs=2))
        gn_small = phase.enter_context(tc.tile_pool(name="gn_small", bufs=4))
        ps_small = phase.enter_context(
            tc.tile_pool(name="ps_small", bufs=2, space="PSUM"))
        silu_args = []
        for ic in range(4):
            xt = gnp.tile([128, L], f32, tag="xin", name=f"x{ic}")
            nc.sync.dma_start(xt[:], t["x"][128 * ic:128 * ic + 128])
            sums = gn_small.tile([128, 2], f32, tag="sums", name=f"sums{ic}")
            nc.vector.tensor_reduce(sums[:, 0:1], xt[:], X, OP.add)
            sq = gnp.tile([128, L], bf16, tag="sqscratch", name=f"sq{ic}")
            nc.scalar.activation(sq[:], xt[:], AT.Square, accum_out=sums[:, 1:2])
            gps = ps_small.tile([128, 128], f32, tag="small", name=f"gps{ic}")
            nc.tensor.matmul(gps[:8, 0:2], g_sb[:], sums[:], start=True, stop=True)
            st = gn_small.tile([8, 6], f32, tag="gnstat", name=f"gnstat{ic}")
            # st cols: 0 mean, 1 ex2, 2 var, 3 sd, 4 rstd
            nc.vector.tensor_scalar(st[:, 0:2], gps[:8, 0:2], 1.0 / 65536.0,
                                    None, OP.mult)
            nc.vector.tensor_tensor(st[:, 2:3], st[:, 0:1], st[:, 0:1], OP.mult)
            nc.vector.tensor_sub(st[:, 2:3], st[:, 1:2], st[:, 2:3])
            nc.scalar.activation(st[:, 3:4], st[:, 2:3], AT.Sqrt,
                                 bias=eps_sb[:8, 0:1])
            nc.vector.reciprocal(st[:, 4:5], st[:, 3:4])
            rhs2 = gn_small.tile([8, 2], f32, tag="gnrhs", name=f"gnrhs{ic}")
            nc.vector.tensor_copy(rhs2[:, 0:1], st[:, 4:5])
            nc.vector.tensor_copy(rhs2[:, 1:2], st[:, 0:1])
            bps = ps_small.tile([128, 128], f32, tag="small", name=f"bps{ic}")
            nc.tensor.matmul(bps[:128, 0:2], gT_sb[:], rhs2[:], start=True, stop=True)
            ab = gn_small.tile([128, 2], f32, tag="gnab", name=f"gnab{ic}")
            nc.vector.tensor_mul(ab[:, 0:1], bps[:128, 0:1], gns_sb[:, ic:ic + 1])
            nc.vector.tensor_mul(ab[:, 1:2], bps[:128, 1:2], ab[:, 0:1])
            nc.vector.tensor_sub(ab[:, 1:2], gnb_sb[:, ic:ic + 1], ab[:, 1:2])
            silu_args.append((ab, xt))
        for ic in range(4):
            ab, xt = silu_args[ic]
            hpv = hpads[ic].rearrange("p (z y x) -> p z y x",
                                      z=PADV, y=PADV, x=PADV)
            sg = gnp.tile([128, L], f32, tag="sg", name=f"sg{ic}")
            nc.scalar.activation(sg[:], xt[:], AT.Sigmoid,
                                 bias=ab[:, 1:2], scale=ab[:, 0:1])
            nc.vector.tensor_scalar(xt[:], xt[:], ab[:, 0:1], ab[:, 1:2],
                                    OP.mult, OP.add)
            nc.vector.tensor_mul(hpv[:, 1:17, 1:17, 1:17], xt[:], sg[:])

        # ---------------- mask prep: rounded px, py -> pflat_d ----------
        mp = phase.enter_context(tc.tile_pool(name="maskprep", bufs=1))
        vpts_sb = mp.tile([3, L], f32)
        nc.sync.dma_start(vpts_sb[:], t["vptsT"])
        pm_sb = mp.tile([3, 3], f32)
        nc.sync.dma_start(pm_sb[:], t["pmat"])
        pcT = mp.tile([128, 96], f32)
        for zt in range(32):
            pps = ps_small.tile([128, 128], f32, tag="small", name=f"pc{zt}")
            nc.tensor.matmul(pps[:, 0:3], vpts_sb[:, 128 * zt:128 * zt + 128],
                             pm_sb[:], start=True, stop=True)
            nc.vector.tensor_copy(pcT[:, 3 * zt:3 * zt + 3], pps[:, 0:3])
        pxy = mp.tile([128, 64], f32)  # cols 0:32 px, 32:64 py
        rec = mp.tile([128, 32], f32)
        nc.vector.reciprocal(rec[:], pcT[:, 2:96:3])
        nc.vector.tensor_mul(pxy[:, 0:32], pcT[:, 0:96:3], rec[:])
        nc.vector.tensor_mul(pxy[:, 32:64], pcT[:, 1:96:3], rec[:])
        nc.vector.tensor_scalar(pxy[:], pxy[:], 0.0, 223.0, OP.max, OP.min)
        # round-to-nearest-even at integer granularity (|x| <= 223 << 2^22)
        nc.vector.tensor_scalar(pxy[:], pxy[:], 12582912.0, 12582912.0,
                                OP.add, OP.subtract)
        for i in range(2):
            tps = ps_small.tile([128, 128], f32, tag="small", name=f"tp{i}")
            nc.tensor.transpose(tps[:32, :], pxy[:, 32 * i:32 * i + 32], ident[:])
            tsb = mp.tile([32, 128], f32, tag="tsb", name=f"tsb{i}")
            nc.vector.tensor_copy(tsb[:], tps[:32, :])
            nc.sync.dma_start(pflat_d[i:i + 1, :], tsb[:])

    # ---------------- K/V projections ----------------
    kvp = ctx.enter_context(tc.tile_pool(name="kv", bufs=1))
    khT = [kvp.tile([128, S], bf16, tag=f"khT{i}", name=f"khT{i}") for i in range(4)]
    vh_aug = [kvp.tile([sz, 1024], bf16, tag=f"vha{i}", name=f"vha{i}")
              for i, (so, sz) in enumerate(SPARTS)]
    with ExitStack() as phase:
        kvt = phase.enter_context(tc.tile_pool(name="kvt", bufs=1))
        kv_ps = phase.enter_context(
            tc.tile_pool(name="kv_ps", bufs=1, space="PSUM"))
        skT = [kvt.tile([128, S], bf16, tag=f"skT{j}", name=f"skT{j}")
               for j in range(6)]
        wk_sb = [kvt.tile([128, F], bf16, tag=f"wk{j}", name=f"wk{j}")
                 for j in range(6)]
        wv_sb = [kvt.tile([128, F], bf16, tag=f"wv{j}", name=f"wv{j}")
                 for j in range(6)]
        for j in range(6):
            nc.sync.dma_start(skT[j][:], t["sketchT"][128 * j:128 * j + 128])
            nc.sync.dma_start(wk_sb[j][:], t["wkT"][128 * j:128 * j + 128])
            nc.sync.dma_start(wv_sb[j][:], t["wvT"][128 * j:128 * j + 128])
        ck_sb = [kvt.tile([128, S], bf16, tag=f"ck{i}", name=f"ck{i}")
                 for i in range(4)]
        for i in range(4):
            nc.sync.dma_start(ck_sb[i][:], t["ck"][128 * i:128 * i + 128])
        cvT_sb = [kvt.tile([sz, F], f32, tag=f"cv{i}", name=f"cv{i}")
                  for i, (so, sz) in enumerate(SPARTS)]
        for i, (so, sz) in enumerate(SPARTS):
            nc.sync.dma_start(cvT_sb[i][:], t["cvT"][so:so + sz])

        for oc in range(4):
            kps = kv_ps.tile([128, S], f32, tag="kps", name=f"kps{oc}")
            for j in range(6):
                nc.tensor.matmul(kps[:], wk_sb[j][:, 128 * oc:128 * oc + 128],
                                 skT[j][:], start=(j == 0), stop=(j == 5))
            nc.vector.tensor_add(khT[oc][:], kps[:], ck_sb[oc][:])
            if DEBUG_DUMPS:
                nc.sync.dma_start(khT_dd[128 * oc:128 * oc + 128], khT[oc][:])

        for i, (so, sz) in enumerate(SPARTS):
            vps = kv_ps.tile([128, F], f32, tag="vps", name=f"vps{i}")
            for j in range(6):
                nc.tensor.matmul(vps[:sz], skT[j][:, so:so + sz], wv_sb[j][:],
                                 start=(j == 0), stop=(j == 5))
            va = vh_aug[i].rearrange("p (h d) -> p h d", h=H, d=128)
            nc.vector.tensor_add(
                va[:, :, 0:64],
                vps[:sz].rearrange("p (h d) -> p h d", h=H, d=64),
                cvT_sb[i][:].rearrange("p (h d) -> p h d", h=H, d=64))
            nc.vector.memset(va[:, :, 64:128], 1.0)
            if DEBUG_DUMPS:
                nc.sync.dma_start(vha_dd[so:so + sz], vh_aug[i][:sz])

    # ---------------- conv (27 shifted matmuls) -> qhT ----------------
    wpool = ctx.enter_context(tc.tile_pool(name="convw", bufs=2))
    cqpool = ctx.enter_context(tc.tile_pool(name="cq", bufs=2))
    conv_ps = ctx.enter_context(tc.tile_pool(name="conv_ps", bufs=2, space="PSUM"))
    cevict = ctx.enter_context(tc.tile_pool(name="cevict", bufs=3))
    offsets = [(dz, dy, dx) for dz in (-1, 0, 1) for dy in (-1, 0, 1)
               for dx in (-1, 0, 1)]
    hpv = [hpads[i].rearrange("p (z y x) -> p z y x", z=PADV, y=PADV, x=PADV)
           for i in range(4)]
    ksl = [(0, KSPLIT), (KSPLIT, 27)]
    for oc in range(4):
        wts = []
        for (k0, k1) in ksl:
            wt = wpool.tile([128, (k1 - k0) * 4 * 128], bf16, tag="wt",
                            name=f"wt{oc}_{k0}")
            nc.sync.dma_start(wt[:], t["wfold"][oc][:, k0 * 512:k1 * 512])
            wts.append(wt)
        cqt = cqpool.tile([128, L], bf16, tag="cqt", name=f"cqt{oc}")
        nc.sync.dma_start(cqt[:], t["cq"][oc])
        for lc in range(NLC):
            cps = conv_ps.tile([128, LC], f32, tag="cps", name=f"cps{oc}_{lc}")
            n = 0
            for half, (k0, k1) in enumerate(ksl):
                wt = wts[half]
                for k in range(k0, k1):
                    dz, dy, dx = offsets[k]
                    for ic in range(4):
                        rhs = hpv[ic][:, 2 * lc + 1 + dz:2 * lc + 3 + dz,
                                      1 + dy:17 + dy, 1 + dx:17 + dx]
                        w = wt[:, ((k - k0) * 4 + ic) * 128:
                               ((k - k0) * 4 + ic) * 128 + 128]
                        nc.tensor.matmul(cps[:], w, rhs,
                                         start=(n == 0), stop=(n == 107))
                        n += 1
            ev = cevict.tile([128, LC], bf16, tag="cev", name=f"cev{oc}_{lc}")
            nc.vector.scalar_tensor_tensor(ev[:], cps[:], 0.0,
                                           cqt[:, LC * lc:LC * lc + LC],
                                           OP.add, OP.add)
            nc.sync.dma_start(qhT_d[128 * oc:128 * oc + 128, LC * lc:LC * lc + LC],
                              ev[:])

    # ---------------- attention ----------------
    ap_m = ctx.enter_context(tc.tile_pool(name="attn_mask", bufs=1))
    ap_q = ctx.enter_context(tc.tile_pool(name="attn_q", bufs=3))
    ap_s = ctx.enter_context(tc.tile_pool(name="attn_s", bufs=1, space="PSUM"))
    ap_o = ctx.enter_context(tc.tile_pool(name="attn_o", bufs=2, space="PSUM"))
    ap_e = ctx.enter_context(tc.tile_pool(name="attn_e", bufs=2))
    ap_r = ctx.enter_context(tc.tile_pool(name="attn_r", bufs=3))
    for lc in range(NLC):
        lsl = slice(LC * lc, LC * lc + LC)
        msk = []
        pb = ap_m.tile([128, 2 * LC], f32, tag="pb", bufs=2, name=f"pb{lc}")
        nc.sync.dma_start(pb[:, 0:LC], pflat_d[0:1, lsl].to_broadcast((128, LC)))
        nc.sync.dma_start(pb[:, LC:], pflat_d[1:2, lsl].to_broadcast((128, LC)))
        for i, (so, sz) in enumerate(SPARTS):
            dxy = ap_m.tile([128, 2 * LC], f32, tag="dxy", name=f"dxy{lc}_{i}")
            nc.vector.tensor_scalar(dxy[:sz, 0:LC], pb[:sz, 0:LC],
                                    cy_sb[:sz, i:i + 1], None, OP.subtract)
            nc.vector.tensor_scalar(dxy[:sz, LC:], pb[:sz, LC:],
                                    cx_sb[:sz, i:i + 1], None, OP.subtract)
            d2 = ap_m.tile([128, LC], f32, tag="d2", name=f"d2{lc}_{i}")
            nc.vector.tensor_mul(d2[:sz], dxy[:sz, 0:LC], dxy[:sz, 0:LC])
            nc.vector.tensor_mul(dxy[:sz, LC:], dxy[:sz, LC:], dxy[:sz, LC:])
            nc.vector.tensor_add(d2[:sz], d2[:sz], dxy[:sz, LC:])
            m = ap_m.tile([128, LC], bf16, tag="m", bufs=4, name=f"m{lc}_{i}")
            nc.vector.tensor_scalar(m[:sz], d2[:sz], 576.0, None, OP.is_lt)
            msk.append(m)
            if DEBUG_DUMPS:
                nc.sync.dma_start(mask_d[128 * i:128 * i + sz, lsl], m[:sz])
        for h in range(H):
            hb = 64 * (h % 2)
            if hb == 0:
                qsl = ap_q.tile([128, LC], bf16, tag="qsl", name=f"q{lc}_{h}")
                nc.sync.dma_start(qsl[:],
                                  qhT_d[64 * h:64 * h + 128, lsl])
            ot = ap_o.tile([128, LC], f32, tag="ot", name=f"ot{lc}_{h}")
            for i, (so, sz) in enumerate(SPARTS):
                sps = ap_s.tile([128, LC], f32, tag=f"sps{i}", name=f"s{lc}_{h}_{i}")
                nc.tensor.matmul(sps[:sz], khT[h // 2][hb:hb + 64, so:so + sz],
                                 qsl[hb:hb + 64, :], start=True, stop=True)
                e = ap_e.tile([128, LC], bf16, tag=f"e{i}", name=f"e{lc}_{h}_{i}")
                nc.scalar.activation(e[:sz], sps[:sz], AT.Exp, scale=0.125)
                em = ap_e.tile([128, LC], bf16, tag=f"em{i}", name=f"em{lc}_{h}_{i}")
                nc.vector.tensor_mul(em[:sz], e[:sz], msk[i][:sz])
                nc.tensor.matmul(ot[:], vh_aug[i][:, 128 * h:128 * h + 128],
                                 em[:sz], start=(i == 0), stop=(i == 1))
            rsb = ap_r.tile([64, LC], f32, tag="rsb", name=f"rsb{lc}_{h}")
            nc.vector.tensor_copy(rsb[:], ot[64:128, :])
            rr = ap_r.tile([64, LC], f32, tag="rr", name=f"rr{lc}_{h}")
            nc.vector.reciprocal_approx_fast(rr[:], rsb[:])
            on = ap_e.tile([64, LC], bf16, tag="on", bufs=3, name=f"on{lc}_{h}")
            nc.vector.tensor_mul(on[:], ot[0:64, :], rr[:])
            nc.sync.dma_start(oT_d[64 * h:64 * h + 64, lsl], on[:])

    # ---------------- output projection ----------------
    op_rhs = ctx.enter_context(tc.tile_pool(name="op_rhs", bufs=2))
    op_ps = ctx.enter_context(tc.tile_pool(name="op_ps", bufs=2, space="PSUM"))
    op_ev = ctx.enter_context(tc.tile_pool(name="op_ev", bufs=3))
    for lc in range(NLC):
        lsl = slice(LC * lc, LC * lc + LC)
        orhs = op_rhs.tile([128, 4 * LC], bf16, tag="orhs", name=f"orhs{lc}")
        for j in range(4):
            nc.sync.dma_start(orhs[:, LC * j:LC * j + LC],
                              oT_d[128 * j:128 * j + 128, lsl])
        for oc in range(4):
            ops = op_ps.tile([128, LC], f32, tag="ops", name=f"ops{lc}_{oc}")
            for j in range(4):
                nc.tensor.matmul(ops[:], wo_sb[j][:, 128 * oc:128 * oc + 128],
                                 orhs[:, LC * j:LC * j + LC],
                                 start=(j == 0), stop=(j == 3))
            oe = op_ev.tile([128, LC], f32, tag="oe", name=f"oe{lc}_{oc}")
            nc.vector.tensor_scalar(oe[:], ops[:], outb_sb[:, oc:oc + 1], None,
                                    OP.add)
            nc.sync.dma_start(t["out"][128 * oc:128 * oc + 128, lsl], oe[:])


DEBUG_DUMPS = False
_CACHE = {}


def get_program():
    key = ("nc", DEBUG_DUMPS)
    if key not in _CACHE:
        nc = bacc.Bacc("TRN2", target_bir_lowering=False, debug=False,
                       enable_asserts=False, num_devices=B)
        tensors = _declare(nc)
        with tile.TileContext(nc) as tc:
            with ExitStack() as ctx:
                _build(ctx, tc, tensors)
        nc.compile()
        _CACHE[key] = nc
    return _CACHE[key]


# ------------------------------------------------------------- host glue
def prep_in_maps(inputs):
    xx = np.asarray(inputs["x"], np.float32)
    sketch = np.asarray(inputs["sketch_feature"], np.float32)
    pmat = np.asarray(inputs["projection_matrix"], np.float32)
    gn_scale = np.asarray(inputs["gn_scale"], np.float32)
    gn_bias = np.asarray(inputs["gn_bias"], np.float32)
    conv_w = np.asarray(inputs["conv_w"], np.float32)
    conv_b = np.asarray(inputs["conv_b"], np.float32)
    k_w = np.asarray(inputs["k_w"], np.float32)
    k_b = np.asarray(inputs["k_b"], np.float32)
    v_w = np.asarray(inputs["v_w"], np.float32)
    v_b = np.asarray(inputs["v_b"], np.float32)
    in_proj_w = np.asarray(inputs["in_proj_w"], np.float32)
    in_proj_b = np.asarray(inputs["in_proj_b"], np.float32)
    out_w = np.asarray(inputs["out_w"], np.float32)
    out_b = np.asarray(inputs["out_b"], np.float32)
    wq, wk, wv = np.split(in_proj_w, 3, 0)
    bq, bk, bv = np.split(in_proj_b, 3, 0)

    # folded conv weights, laid out partition-major per oc tile:
    # (oc_t, ic128, k, ic_t, oc128)
    wfold = np.einsum("qc,cikab->qikab", wq, conv_w)        # (512q, 512c, 3,3,3)
    wfold = wfold.reshape(4, 128, 4, 128, 27)               # qt, q, ct, c, k
    wfold = wfold.transpose(0, 3, 4, 2, 1)                  # qt, c128, k, ct, q128
    wfold = np.ascontiguousarray(wfold.reshape(4, 128, 27 * 4 * 128)).astype(BF16)
    cq = (wq @ VOXEL_PE.T + (wq @ conv_b)[:, None] + bq[:, None])
    cq = np.ascontiguousarray(cq.reshape(4, 128, L)).astype(BF16)
    wkT = np.ascontiguousarray((wk @ k_w).T).astype(BF16)   # (768, 512)
    ck = (wk @ COND_PE.T + (wk @ k_b)[:, None] + bk[:, None]).astype(BF16)
    wvT = np.ascontiguousarray((wv @ v_w).T).astype(BF16)
    cvT = np.ascontiguousarray(
        (wv @ COND_PE.T + (wv @ v_b)[:, None] + bv[:, None]).T).astype(np.float32)
    woT = np.ascontiguousarray(out_w.T).astype(BF16)

    gmat = np.zeros((128, 8), np.float32)
    gmat[np.arange(128), np.arange(128) // 16] = 1.0
    gmatT = np.ascontiguousarray(gmat.T)

    shared = {
        "wfold": wfold, "cq": cq, "wkT": wkT, "wvT": wvT, "ck": ck, "cvT": cvT,
        "woT": woT,
        "outb": out_b.reshape(F, 1).copy(),
        "gns": gn_scale.reshape(F, 1).copy(),
        "gnb": gn_bias.reshape(F, 1).copy(),
        "vptsT": np.ascontiguousarray(VOXEL_PTS.T),
        "cx16": CX16.reshape(S, 1).copy(), "cy16": CY16.reshape(S, 1).copy(),
        "gmat": gmat, "gmatT": gmatT,
    }
    in_maps = []
    for b in range(B):
        m = dict(shared)
        m["x"] = np.ascontiguousarray(xx[b].reshape(F, L))
        m["sketchT"] = np.ascontiguousarray(sketch[b].T).astype(BF16)
        m["pmat"] = np.ascontiguousarray(pmat[b, 0].T)
        in_maps.append(m)
    return in_maps


def _ensure_ntff_hook():
    import sys
    import types
    try:
        import antenv.axon_hooks  # noqa: F401
        return
    except ImportError:
        pass
    try:
        sys.path.insert(0, "/root/.axon_site/trn_agent_boot")
        import trn_boot
        hook = trn_boot._ntff_profile_via_ctypes("/opt/axon/libaxon_pjrt.so")
        mod = types.ModuleType("antenv.axon_hooks")
        holder = [hook]
        mod.set_axon_ntff_profile_hook = lambda h: holder.__setitem__(0, h)
        mod.get_axon_ntff_profile_hook = lambda: holder[0]
        sys.modules["antenv.axon_hooks"] = mod
        import antenv
        antenv.axon_hooks = mod
    except Exception as e:  # degrade to no-trace
        print("ntff hook setup failed:", e)


def run(inputs, trace=False, tmpdir=None):
    from concourse.bass_utils import run_bass_kernel_spmd
    if trace:
        _ensure_ntff_hook()
    nc = get_program()
    in_maps = prep_in_maps(inputs)
    res = run_bass_kernel_spmd(nc, in_maps, list(range(B)), trace=trace,
                               tmpdir=tmpdir)
    out = np.stack([np.asarray(res.results[i]["out"], np.float32)
                    for i in range(B)])
    return out.reshape(B, F, IMG, IMG, IMG), res


def kernel(**inputs):
    out, _ = run(inputs)
    return out

